# revision 2
# baseline (speedup 1.0000x reference)
"""MoE feed-forward Trainium2 kernel, v3.1: expert-parallel, host routing.

8 cores x 2 slot-blocks; each block processes a contiguous chunk of one
expert's routed tokens with that expert's weights (per-block weight
inputs), so per-core load is balanced to ceil(total_chunks/8) 128-slot
chunks regardless of routing skew.  The host computes exact fp32
routing, stages each block's tokens in the transposed [ki, ko, slot]
layout, and sums the two expert contributions per token at the end.

Device pipeline per block (branch-free, compiled per block-cap pair):
  stream xT + weight tiles -> gate/up matmuls (fp32 PSUM) -> silu*up
  (bf16 actT) -> down-proj matmuls -> gate-scale -> y rows out (bf16).
"""

import sys

sys.path.insert(0, "/opt/trn_rl_repo")

import numpy as np
import ml_dtypes

import concourse.bass as bass
import concourse.bacc as bacc
import concourse.tile as tile
import concourse.mybir as mybir
from concourse.bass import ts, ds

F32 = mybir.dt.float32
BF16 = mybir.dt.bfloat16
OP = mybir.AluOpType
ACT = mybir.ActivationFunctionType

N_CORES = 8
B, T, H, I, E = 4, 4096, 1024, 2048, 8
BT = B * T
KH = H // 128   # 8
KI = I // 128   # 16


def _split_blocks(C):
    """Split C (multiple of 128) into blocks <= 2304, preferring 2048."""
    n_full = C // 2048
    rem = C - n_full * 2048
    blocks = [2048] * n_full
    if rem:
        if blocks and rem + 2048 <= 2304:
            blocks[-1] += rem
        else:
            blocks.append(rem)
    return blocks


def _split_n(cap):
    """512-column matmul chunks over a block (last one ragged)."""
    out = []
    n0 = 0
    while n0 < cap:
        out.append((n0, min(512, cap - n0)))
        n0 += 512
    return out


def build_program(caps, debug=False):
    """caps: per-block slot capacities (multiples of 128), e.g. (2048, 2176)."""
    NM = sum(caps) // 128

    nc = bacc.Bacc("TRN2", target_bir_lowering=False, debug=debug)

    xt_d, wg_d, wu_d, wd_d = [], [], [], []
    for b, cap in enumerate(caps):
        xt_d.append(nc.dram_tensor(f"xt{b}", [128, KH, cap], BF16,
                                   kind="ExternalInput"))
        wg_d.append(nc.dram_tensor(f"wgt{b}", [KI, 128, KH, 128], BF16,
                                   kind="ExternalInput"))
        wu_d.append(nc.dram_tensor(f"wut{b}", [KI, 128, KH, 128], BF16,
                                   kind="ExternalInput"))
        wd_d.append(nc.dram_tensor(f"wd{b}", [128, KI, H], BF16,
                                   kind="ExternalInput"))
    gates_d = nc.dram_tensor("gates", [128, NM], F32, kind="ExternalInput")
    y_d = nc.dram_tensor("y", [sum(caps), H], BF16, kind="ExternalOutput")

    with tile.TileContext(nc) as tc:
        with (
            tc.tile_pool(name="const", bufs=1) as pconst,
            tc.tile_pool(name="xt", bufs=1) as pxt,
            tc.tile_pool(name="act", bufs=1) as pact,
            tc.tile_pool(name="wgt", bufs=2) as pwgt,
            tc.tile_pool(name="wdp", bufs=2) as pwdp,
            tc.tile_pool(name="ev", bufs=2) as pev,
            tc.tile_pool(name="yp", bufs=3) as pyp,
            tc.tile_pool(name="gups", bufs=3, space="PSUM") as pgu,
            tc.tile_pool(name="yps", bufs=2, space="PSUM") as pyps,
        ):
            gates_sb = pconst.tile([128, NM], F32)
            nc.sync.dma_start(gates_sb[:], gates_d[:])
            zw_sb = pconst.tile([128, 128], BF16)
            nc.vector.memset(zw_sb[:], 0.0)
            warm_sb = pconst.tile([128, 512], BF16)
            nc.vector.memset(warm_sb[:], 0.0)

            # block 0 xT streamed in column chunks (first chunk small so
            # the first matmul group starts ~2us in); wd0/wd1 follow on
            # the gpsimd ring.
            xt0_sb = pxt.tile([128, KH, caps[0]], BF16, tag="xt")
            chunks0 = [(0, 256), (256, 256)] + _split_n(caps[0])[1:]
            for c0, csz in chunks0:
                nc.gpsimd.dma_start(
                    xt0_sb[:, :, ds(c0, csz)], xt_d[0][:, :, ds(c0, csz)]
                )

            # HAM warm-up while the first chunks stream in (covers the
            # xt0 landing window so the PE never sits idle-cold)
            ps_w = pgu.tile([128, 512], F32, tag="psg")
            for _ in range(26):
                nc.tensor.matmul(ps_w[:], lhsT=zw_sb[:], rhs=warm_sb[:],
                                 start=True, stop=True)

            wd_sb = []
            for b in range(len(caps)):
                wd_t = pwdp.tile([128, KI, H], BF16, tag="wd")
                wd_sb.append(wd_t)
                for k4 in range(0, KI, 4):
                    nc.gpsimd.dma_start(wd_t[:, ds(k4, 4), :],
                                        wd_d[b][:, ds(k4, 4), :])

            m_base = 0
            xt_b = xt0_sb
            for b, cap in enumerate(caps):
                actT = pact.tile([128, KI, cap], BF16, tag="actT")
                nblocks = _split_n(cap)
                for ic in range(KI):
                    wg_t = pwgt.tile([128, KH, 128], BF16, tag="wg")
                    nc.sync.dma_start(wg_t[:], wg_d[b][ic])
                    wu_t = pwgt.tile([128, KH, 128], BF16, tag="wu")
                    nc.sync.dma_start(wu_t[:], wu_d[b][ic])
                    for n0, nsz in nblocks:
                        ps_g = pgu.tile([128, 512], F32, tag="psg")
                        ps_u = pgu.tile([128, 512], F32, tag="psu")
                        for k in range(KH):
                            nc.tensor.matmul(
                                ps_g[:, :nsz], lhsT=wg_t[:, k, :],
                                rhs=xt_b[:, k, ds(n0, nsz)],
                                start=(k == 0), stop=(k == KH - 1),
                            )
                        for k in range(KH):
                            nc.tensor.matmul(
                                ps_u[:, :nsz], lhsT=wu_t[:, k, :],
                                rhs=xt_b[:, k, ds(n0, nsz)],
                                start=(k == 0), stop=(k == KH - 1),
                            )
                        s_sb = pev.tile([128, 512], F32, tag="s")
                        nc.scalar.activation(s_sb[:, :nsz], ps_g[:, :nsz],
                                             ACT.Silu)
                        nc.vector.tensor_tensor(
                            actT[:, ic, ds(n0, nsz)], s_sb[:, :nsz],
                            ps_u[:, :nsz], op=OP.mult,
                        )
                # next block's xT on the sync ring (after this block's
                # weights, before the next block's); the WAR dependency
                # on this block's last gate/up read is resolved exactly
                # when the down phase starts.
                if b + 1 < len(caps):
                    xt_next = pxt.tile([128, KH, caps[b + 1]], BF16, tag="xt")
                    for k2 in range(0, KH, 2):
                        nc.sync.dma_start(
                            xt_next[:, ds(k2, 2), :],
                            xt_d[b + 1][:, ds(k2, 2), :],
                        )

                for m in range(cap // 128):
                    y_sb = pyp.tile([128, H], BF16, tag="y")
                    for hb in range(2):
                        ps_y = pyps.tile([128, 512], F32, tag="psy")
                        for k in range(KI):
                            nc.tensor.matmul(
                                ps_y[:],
                                lhsT=actT[:, k, ts(m, 128)],
                                rhs=wd_sb[b][:, k, ts(hb, 512)],
                                start=(k == 0), stop=(k == KI - 1),
                            )
                        nc.scalar.mul(
                            y_sb[:, ts(hb, 512)], ps_y[:],
                            mul=gates_sb[:, m_base + m : m_base + m + 1],
                        )
                    nc.scalar.dma_start(
                        y_d[ds((m_base + m) * 128, 128), :], y_sb[:]
                    )
                m_base += cap // 128
                if b + 1 < len(caps):
                    xt_b = xt_next

    nc.compile()
    return nc


# ======================= host staging =================================

_PROGRAM_CACHE = {}


def _route(h, gate_w):
    """Exact fp32 routing: top-2 experts + renormalized gates."""
    L = h @ gate_w.T                                   # [BT, E]
    top2 = np.argpartition(-L, 2, axis=1)[:, :2]
    l2 = np.take_along_axis(L, top2, axis=1)
    order = np.argsort(-l2, axis=1)
    top2 = np.take_along_axis(top2, order, axis=1)
    m = L.max(axis=1, keepdims=True)
    p = np.exp(L - m)
    p /= p.sum(axis=1, keepdims=True)
    g = np.take_along_axis(p, top2, axis=1)
    g = g / np.maximum(g.sum(axis=1, keepdims=True), 1e-9)
    return top2, g.astype(np.float32)


def _try_balanced(n_chunks, cc):
    """Try to pack each expert's chunks into 16 single-expert parts
    (8 of size c1 + 8 of c2, c1+c2=cc).  Returns (caps, parts) with
    parts[(core, blk)] = (expert, chunk_start, n), or None."""
    c1, c2 = cc // 2, cc - cc // 2
    sizes = [c1] * N_CORES + [c2] * N_CORES
    exps = sorted(range(E), key=lambda e: -n_chunks[e])
    assign = [None] * 16
    free = list(range(16))
    for e in exps:
        rem = int(n_chunks[e])
        start = 0
        while rem > 0:
            best = None
            for i in free:
                waste = max(0, sizes[i] - rem)
                if best is None or waste < best[1]:
                    best = (i, waste)
            if best is None:
                return None
            i = best[0]
            free.remove(i)
            take = min(sizes[i], rem)
            assign[i] = (e, start, take)
            start += take
            rem -= take
    parts = {}
    for i in range(16):
        core, blk = i % N_CORES, i // N_CORES
        parts[(core, blk)] = assign[i] if assign[i] else (-1, 0, 0)
    return (c1 * 128, c2 * 128), parts


def _partition(n_chunks):
    """Returns (caps, parts): caps = per-block slot capacities (same on
    every core), parts[(core, blk)] = (expert, chunk_start, n_chunks)."""
    total = int(sum(n_chunks))
    base = -(-total // N_CORES)
    for cc in range(base, base + 4):
        if cc // 2 <= 18 and (cc - cc // 2) <= 18:
            r = _try_balanced(n_chunks, cc)
            if r is not None:
                return r
    # fallback: pure expert-per-core, padded to the max expert
    Cmax = max(128, int(max(n_chunks)) * 128)
    caps = tuple(_split_blocks(Cmax))
    parts = {}
    for core in range(N_CORES):
        start = 0
        for blk, cap in enumerate(caps):
            take = min(cap // 128, max(0, int(n_chunks[core]) - start))
            parts[(core, blk)] = (core, start, take) if take else (-1, 0, 0)
            start += take
    return caps, parts


def _stage(hidden_states, gate_w, wg, wu, wd):
    bf = ml_dtypes.bfloat16
    h = np.asarray(hidden_states, dtype=np.float32).reshape(-1, H)
    gate_w = np.asarray(gate_w, dtype=np.float32)
    top2, gates2 = _route(h, gate_w)

    tok_e, gat_e = [], []
    for e in range(E):
        rows, cols = np.nonzero(top2 == e)
        tok_e.append(rows)
        gat_e.append(gates2[rows, cols])
    n_chunks = [-(-len(t) // 128) for t in tok_e]
    caps, parts = _partition(n_chunks)
    CT = sum(caps)
    NM = CT // 128

    hbf = h.astype(bf)

    def _retile_gu(w):
        w = np.asarray(w, dtype=np.float32).astype(bf)
        return np.ascontiguousarray(
            w.reshape(KH, 128, KI, 128).transpose(2, 1, 0, 3))

    def _retile_d(w):
        w = np.asarray(w, dtype=np.float32).astype(bf)
        return np.ascontiguousarray(w.reshape(KI, 128, H).transpose(1, 0, 2))

    wg_host = {}
    wu_host = {}
    wd_host = {}

    in_maps = [dict() for _ in range(N_CORES)]
    gcols = [np.zeros((128, NM), np.float32) for _ in range(N_CORES)]
    # Y row base per part for the combine
    yrow_of = [np.empty(len(t), np.int64) for t in tok_e]

    cap_base = np.concatenate([[0], np.cumsum(caps)])
    for (core, blk), (e, cstart, nch) in parts.items():
        cap = caps[blk]
        im = in_maps[core]
        if e < 0 or nch == 0:
            im[f"xt{blk}"] = np.zeros((128, KH, cap), bf)
            im[f"wgt{blk}"] = wg_host.setdefault(0, _retile_gu(wg[0]))
            im[f"wut{blk}"] = wu_host.setdefault(0, _retile_gu(wu[0]))
            im[f"wd{blk}"] = wd_host.setdefault(0, _retile_d(wd[0]))
            continue
        s0 = cstart * 128
        s1 = min(s0 + nch * 128, len(tok_e[e]))
        n = s1 - s0
        toks = tok_e[e][s0:s1]
        xT = np.zeros((128, KH, cap), bf)
        xT[:, :, :n] = hbf[toks].reshape(n, KH, 128).transpose(2, 1, 0)
        im[f"xt{blk}"] = xT
        im[f"wgt{blk}"] = wg_host.setdefault(e, _retile_gu(wg[e]))
        im[f"wut{blk}"] = wu_host.setdefault(e, _retile_gu(wu[e]))
        im[f"wd{blk}"] = wd_host.setdefault(e, _retile_d(wd[e]))
        base = int(cap_base[blk])
        sl = np.arange(n)
        gcols[core][sl % 128, base // 128 + sl // 128] = gat_e[e][s0:s1]
        yrow_of[e][s0:s1] = core * CT + base + sl

    for core in range(N_CORES):
        in_maps[core]["gates"] = gcols[core]

    pos = np.empty((BT, 2), np.int64)
    for e in range(E):
        rows, cols = np.nonzero(top2 == e)
        pos[rows, cols] = yrow_of[e]
    return caps, in_maps, pos


def _combine(results, pos):
    Y = np.concatenate(
        [np.asarray(results[c]["y"]).astype(np.float32)
         for c in range(N_CORES)], axis=0)
    return (Y[pos[:, 0]] + Y[pos[:, 1]]).reshape(B, T, H)


def run(hidden_states, gate_w, wg, wu, wd, trace=False, trace_kwargs=None):
    from concourse.bass_utils import run_bass_kernel_spmd

    caps, in_maps, pos = _stage(hidden_states, gate_w, wg, wu, wd)
    if caps not in _PROGRAM_CACHE:
        _PROGRAM_CACHE[caps] = build_program(caps)
    nc = _PROGRAM_CACHE[caps]
    res = run_bass_kernel_spmd(
        nc, in_maps, core_ids=list(range(N_CORES)),
        trace=trace, **(trace_kwargs or {}),
    )
    return _combine(res.results, pos), res


def kernel(hidden_states, gate_w, wg, wu, wd):
    out, _ = run(hidden_states, gate_w, wg, wu, wd)
    return out
